# revision 1
# baseline (speedup 1.0000x reference)
"""AttentionProtoNet pooling kernel for 8x TRN2 NeuronCores.

reference (per sample of B=64, L=512, H=768):
    upsilon = tanh(hs @ W_fc.T + b_fc)        [L, H]
    nu      = upsilon @ W_nu                  [L]
    alphas  = softmax(nu)                     [L]
    pooled  = alphas @ hs                     [H]

Strategy: data-parallel over B (8 samples per core). Host ships per-core
X^T = hs.reshape(4096, H).T as float32r (tf32-style 10-bit-mantissa
rounding) so the TensorEngine runs the big [4096x768]x[768x768] matmul at
1 cycle/row with near-fp32 accuracy. tanh runs on ACT straight out of
PSUM (per-partition bias), nu is a bf16 matmul against the tanh output,
softmax on 1 partition, alphas broadcast across partitions via GpSimd,
and the pooled weighted sum runs on the VectorEngine as
scalar_tensor_tensor with per-partition accumulate, reading the same
resident X^T tiles. Outputs drain per-sample through a tiny PE transpose.
"""

import sys

sys.path.insert(0, "/opt/trn_rl_repo")

import numpy as np
import ml_dtypes

B, L, H = 64, 512, 768
NCORES = 8
SPC = B // NCORES            # samples per core
TOK = SPC * L                # tokens per core
HC = H // 128                # 128-partition chunks of H
WARMUP_MM = 14               # junk matmuls to lift HAM to K=8/8 during DMA

_compiled = {}


def _round_f32r(a: np.ndarray) -> np.ndarray:
    """Round-to-nearest-even to 10-bit mantissa (drop 13 bits) — the
    pre-rounding TRN2's float32r matmul requires of its inputs."""
    bits = np.ascontiguousarray(a, np.float32).view(np.uint32)
    lsb = (bits >> 13) & np.uint32(1)
    out = (bits + np.uint32(0x0FFF) + lsb) & np.uint32(0xFFFFE000)
    return out.view(np.float32)


def _build():
    import concourse.bass as bass
    import concourse.bacc as bacc
    import concourse.tile as tile
    from concourse import mybir
    from concourse.masks import make_identity

    F32 = mybir.dt.float32
    F32R = mybir.dt.float32r
    BF16 = mybir.dt.bfloat16
    AF = mybir.ActivationFunctionType
    ALU = mybir.AluOpType
    AX = mybir.AxisListType

    nc = bacc.Bacc(None, target_bir_lowering=False)

    xt_d = nc.dram_tensor("xt", [H, TOK], F32R, kind="ExternalInput")
    wt_d = nc.dram_tensor("wt", [H, H], F32R, kind="ExternalInput")
    bfc_d = nc.dram_tensor("bfc", [128, HC], F32, kind="ExternalInput")
    wnu_d = nc.dram_tensor("wnu", [128, HC], BF16, kind="ExternalInput")
    out_d = nc.dram_tensor("out", [SPC, H], F32, kind="ExternalOutput")
    junk_d = nc.dram_tensor("junk", [128, 8], F32)   # warmup sink

    with tile.TileContext(nc) as tc:
        with tc.tile_pool(name="xp", bufs=1) as xp, \
             tc.tile_pool(name="wp", bufs=1) as wp, \
             tc.tile_pool(name="cst", bufs=1) as cst, \
             tc.tile_pool(name="ups", bufs=2) as upsp, \
             tc.tile_pool(name="sm", bufs=2) as smp, \
             tc.tile_pool(name="outp", bufs=2) as outp, \
             tc.tile_pool(name="mmps", bufs=5, space="PSUM") as mmps, \
             tc.tile_pool(name="nups", bufs=2, space="PSUM") as nups, \
             tc.tile_pool(name="tps", bufs=1, space="PSUM") as tps:

            # ---- PE warmup: junk matmuls with no DMA dependency keep the
            # HAM activity window busy while the first tiles stream in.
            wu_sb = cst.tile([128, 512], BF16)
            nc.vector.memset(wu_sb[:], 1.0)
            wu_ps = tps.tile([128, 512], F32, tag="tp", name="wu_ps")
            for i in range(WARMUP_MM):
                nc.tensor.matmul(wu_ps[:], wu_sb[:, 0:128], wu_sb[:],
                                 start=(i == 0), stop=(i == WARMUP_MM - 1))
            wu_out = cst.tile([128, 8], F32)
            nc.scalar.copy(wu_out[:], wu_ps[:, 0:8])
            nc.sync.dma_start(junk_d[:], wu_out[:])

            # ---- weights: k=0 chunks first so sample 0 can start early
            wt_sb = []
            for h in range(HC):
                t = wp.tile([128, H], F32R, tag=f"wt{h}", name=f"wt{h}")
                wt_sb.append(t)
            bfc_sb = cst.tile([128, HC], F32)
            nc.sync.dma_start(bfc_sb[:], bfc_d[:])
            wnu_sb = cst.tile([128, HC], BF16)
            nc.sync.dma_start(wnu_sb[:], wnu_d[:])
            ident = cst.tile([128, 128], F32)
            make_identity(nc, ident[:])

            # ---- X^T resident tiles; sample-0 blocks first
            xt_sb = [xp.tile([128, TOK], F32R, tag=f"xt{h}", name=f"xt{h}")
                     for h in range(HC)]
            dmaeng = [nc.sync, nc.scalar, nc.gpsimd]

            def xt_load(s, h):
                c0, c1 = s * L, (s + 1) * L
                eng = dmaeng[0]
                eng.dma_start(xt_sb[h][:, c0:c1],
                              xt_d[h * 128:(h + 1) * 128, c0:c1])

            for h in range(HC):
                nc.sync.dma_start(wt_sb[h][:, 0:128],
                                  wt_d[h * 128:(h + 1) * 128, 0:128])
                xt_load(0, h)
            # remaining weights + samples, interleaved so sample 0's later
            # k-chunks and sample 1 arrive before the PE needs them
            for h in range(HC):
                nc.sync.dma_start(wt_sb[h][:, 128:H],
                                  wt_d[h * 128:(h + 1) * 128, 128:H])
            for s in range(1, SPC):
                for h in range(HC):
                    xt_load(s, h)

            # ---- per-sample pipeline
            for s in range(SPC):
                c0, c1 = s * L, (s + 1) * L
                ups = upsp.tile([128, HC * L], BF16, tag="ups")
                for k in range(HC):
                    ps = mmps.tile([128, L], F32, tag="mm")
                    for h in range(HC):
                        nc.tensor.matmul(
                            ps[:],
                            wt_sb[h][:, k * 128:(k + 1) * 128],
                            xt_sb[h][:, c0:c1],
                            start=(h == 0),
                            stop=(h == HC - 1),
                        )
                    nc.scalar.activation(
                        ups[:, k * L:(k + 1) * L], ps[:], AF.Tanh,
                        bias=bfc_sb[:, k:k + 1],
                    )

                nu = nups.tile([1, L], F32, tag="nu")
                for k in range(HC):
                    nc.tensor.matmul(
                        nu[:],
                        wnu_sb[:, k:k + 1],
                        ups[:, k * L:(k + 1) * L],
                        start=(k == 0),
                        stop=(k == HC - 1),
                    )

                # softmax over the 512 logits (single partition); nu is
                # small enough that exp() needs no max subtraction
                ex = smp.tile([1, L], F32, tag="ex")
                z = smp.tile([1, 1], F32, tag="z")
                nc.scalar.activation(ex[:], nu[:], AF.Exp, accum_out=z[:])
                rz = smp.tile([1, 1], F32, tag="rz")
                nc.vector.reciprocal(rz[:], z[:])

                # broadcast unnormalized E (recip runs in parallel), pool,
                # then scale pooled by 1/Z
                ab = smp.tile([128, L], F32, tag="ab")
                nc.gpsimd.partition_broadcast(ab[:], ex[:])
                rzb = smp.tile([HC, 1], F32, tag="rzb")
                nc.gpsimd.partition_broadcast(rzb[:], rz[:], channels=HC)
                pooled_u = outp.tile([128, HC], F32, tag="pooled_u")
                for h in range(HC):
                    trash = smp.tile([128, L], F32, tag="trash")
                    nc.vector.scalar_tensor_tensor(
                        trash[:],
                        xt_sb[h][:, c0:c1].bitcast(F32),
                        1.0,
                        ab[:],
                        ALU.mult,
                        ALU.mult,
                        accum_out=pooled_u[:, h:h + 1],
                    )


                # pooled^T [128, HC] -> [HC, 128] -> DRAM row s
                tp = tps.tile([HC, 128], F32, tag="tp")
                nc.tensor.transpose(tp[:], pooled_u[:], ident[:])
                orow = outp.tile([HC, 128], F32, tag="orow")
                nc.scalar.activation(orow[:], tp[:], AF.Copy, scale=rzb[:, 0:1])
                nc.sync.dma_start(
                    out_d[s:s + 1, :].rearrange("o (c p) -> (o c) p", p=128),
                    orow[:],
                )

    nc.finalize()
    return nc


def kernel(hidden_states, W_fc, b_fc, W_nu, _trace=False, _trace_kwargs=None):
    from concourse.bass_utils import run_bass_kernel_spmd

    hs = np.ascontiguousarray(hidden_states, dtype=np.float32)
    W_fc = np.asarray(W_fc, np.float32)
    b_fc = np.asarray(b_fc, np.float32)
    W_nu = np.asarray(W_nu, np.float32)

    wt_host = _round_f32r(W_fc.T)                      # [H(h), H(kout)]
    bfc_host = np.ascontiguousarray(b_fc.reshape(HC, 128).T, np.float32)
    wnu_host = np.ascontiguousarray(
        W_nu.reshape(HC, 128).T.astype(ml_dtypes.bfloat16))

    in_maps = []
    for c in range(NCORES):
        xt = _round_f32r(
            np.ascontiguousarray(
                hs[c * SPC:(c + 1) * SPC].reshape(TOK, H).T))
        in_maps.append(
            {"xt": xt, "wt": wt_host, "bfc": bfc_host, "wnu": wnu_host})

    if "nc" not in _compiled:
        _compiled["nc"] = _build()
    res = run_bass_kernel_spmd(
        _compiled["nc"], in_maps, list(range(NCORES)),
        trace=_trace, **(_trace_kwargs or {}),
    )
    kernel.last_results = res
    out = np.concatenate([np.asarray(r["out"], np.float32) for r in res.results])
    return out

